# revision 27
# baseline (speedup 1.0000x reference)
"""Sliding-window GQA attention (RoPE + tanh soft-cap) on 8 Trainium2 cores.

Sharding: core c = 4*b + hh with b = batch, hh = head-quarter. Each core
handles batch b, q-heads [4*hh, 4*hh+4), kv-heads [2*hh, 2*hh+2) — one
head-group of (4 q-heads, 2 kv-heads); the host sums the 4 partials per batch.

Phases per core:
  A1: q^T = (q_w^T @ x^T), RoPE              -> SBUF qT_sb [128, 8, 2048] f16
  A2: k^T = (k_w^T @ x^T), RoPE              -> SBUF [512, 2048] f16
  A3: v   = (x @ v_w)                        -> SBUF [2048, 512] bf16
  B1: transposed-band attention              -> enc_sb rows [0, 8)
  B2: out = enc^T.T @ o_w over 8 row-tiles   -> DRAM [2048, 3584] partial.

Ring assignment: sync = qw + x stream + output stage; scalar = tables,
sum-row spill, ow loads; gpsimd = kw/vw prefetch (overlaps A1) + sum-row
broadcast. The per-head normalization runs on GpSimd so the slow
DRAM-broadcast round trip never blocks the Vector FIFO. B1 interleaves the
two q-heads of each kv head so one head's softcap/exp latency hides under
the other head's matmuls.
"""

import numpy as np

B, T, D, N, KH, H = 2, 2048, 3584, 16, 8, 256
WINDOW = 1024
SOFT_CAP = 50.0
SCALAR = 0.0625
BASE = 10000.0
NEG = -30000.0  # tanh-domain mask (fits fp16; exp(50*NEG) underflows to 0)

P = 128
NH = 4         # q heads per core
NKH = 2        # kv heads per core
KT = D // P    # 28 contraction tiles
NA = NH * (H // P)   # 8 q^T row-tiles per core
KA = NKH * (H // P)  # 4 k^T row-tiles per core
TB = T // P    # 16 query blocks
TW = 256       # query-pair width for the transposed-attention phase
NCORES = 8
NCH_H = 8      # projection t-chunks (host mirror of NCH)
CH_H = 256

_PROG_CACHE = {}


def _build_program():
    import concourse.bacc as bacc
    import concourse.tile as tile
    import concourse.mybir as mybir

    F32 = mybir.dt.float32
    F16 = mybir.dt.float16
    BF16 = mybir.dt.bfloat16
    Tanh = mybir.ActivationFunctionType.Tanh
    Exp = mybir.ActivationFunctionType.Exp

    nc = bacc.Bacc("TRN2", target_bir_lowering=False, debug=False,
                   num_devices=NCORES)

    CH = 256             # t-chunk for projections
    NCH = T // CH        # 8

    # All DRAM operands are laid out so each SBUF partition's slice is one
    # contiguous DRAM run (HWDGE descriptor generation is the DMA
    # bottleneck otherwise: a [D,T]-layout x gives 512B runs, ~900
    # descriptors and 8-15us per tile load).
    xq = nc.dram_tensor("xq", [NCH, P, KT, CH], F16, kind="ExternalInput")
    qw = nc.dram_tensor("qw", [P, KT, NH * H], F16, kind="ExternalInput")
    kw = nc.dram_tensor("kw", [P, KT, NKH * H], F16, kind="ExternalInput")
    vw = nc.dram_tensor("vw", [P, KT, NKH * H], F16, kind="ExternalInput")
    ow = nc.dram_tensor("ow", [P, D // 512, NA, 512], BF16,
                        kind="ExternalInput")
    cosT = nc.dram_tensor("cosT", [P, T], F16, kind="ExternalInput")
    sinT = nc.dram_tensor("sinT", [P, T], F16, kind="ExternalInput")
    maskt = nc.dram_tensor("maskt", [4 * P, TW], F16, kind="ExternalInput")
    out_p = nc.dram_tensor("out_p", [TB, P, D], F16,
                           kind="ExternalOutput")

    def rope_all(vec, dst, src, cs, sn, tmp_pool, npair):
        # all pairs at once via strided even/odd views:
        # even' = even*cos - odd*sin ; odd' = odd*cos + even*sin
        sv = src.rearrange("p (a x) c -> p x a c", x=2)
        dv = dst.rearrange("p (a x) c -> p x a c", x=2)
        se, so = sv[:, 0], sv[:, 1]
        de, do = dv[:, 0], dv[:, 1]
        csb = cs.unsqueeze(1).broadcast_to([P, npair, CH])
        snb = sn.unsqueeze(1).broadcast_to([P, npair, CH])
        t1 = tmp_pool.tile([P, NA // 2, CH], F16, tag="rt1",
                           name="t1")[:, :npair]
        t2 = tmp_pool.tile([P, NA // 2, CH], F16, tag="rt2",
                           name="t2")[:, :npair]
        vec.tensor_mul(t1, se, csb)
        vec.tensor_mul(t2, so, snb)
        vec.tensor_sub(de, t1, t2)
        # in-order DVE execution makes the tmp reuse safe
        vec.tensor_mul(t1, so, csb)
        vec.tensor_mul(t2, se, snb)
        vec.tensor_add(do, t1, t2)

    import concourse.bass as bass_mod

    with tile.TileContext(nc) as tc:
        with tc.tile_pool(name="p_tab", bufs=1) as p_tab, \
             tc.tile_pool(name="p_perm", bufs=1) as p_perm:
            # tables ride the scalar HWDGE ring: they must not queue ahead
            # of the weight/x loads on the sync ring.
            cos_sb = p_tab.tile([P, T], F16)
            sin_sb = p_tab.tile([P, T], F16)
            nc.scalar.dma_start(cos_sb[:], cosT.ap())
            nc.scalar.dma_start(sin_sb[:], sinT.ap())
            mk_sb = p_tab.tile([P, 4, TW], F16)
            nc.scalar.dma_start(mk_sb[:], maskt.ap().rearrange(
                "(m p) c -> p m c", p=P))
            bias_c = p_tab.tile([P, 1], F32)
            nc.vector.memset(bias_c[:], -10.0)
            # all-ones [128,128] stationary: the key-sum matmul then lands
            # the per-query sums replicated across all 128 partitions, so
            # normalization needs no DRAM broadcast round trip.
            ones_sb = p_tab.tile([P, P], BF16)
            nc.vector.memset(ones_sb[:], 1.0)

            qT_sb = p_perm.tile([P, NA, T], F16)        # 32 KB/part
            kT_sb = p_perm.tile([P, KA, T], F16)        # 16 KB/part
            v_sb = p_perm.tile([P, TB, NKH * H], BF16)  # 16 KB/part

            with tc.tile_pool(name="p_wkv", bufs=1) as p_wkv, \
                 tc.tile_pool(name="ps_a", bufs=2, space="PSUM") as ps_a, \
                 tc.tile_pool(name="p_rope", bufs=2) as ropep:
                # k/v weights prefetch during A1 on the scalar ring (queued
                # behind the small tables), into SBUF disjoint from qw so
                # there is no WAR delay at the A1->A2 boundary and no SDMA
                # contention with the sync ring's critical qw/x loads.
                kw_sb = p_wkv.tile([P, KT, NKH * H], F16, tag="kw")
                vw_sb = p_wkv.tile([P, KT, NKH * H], F16, tag="vw")
                XQ = KT // 4

                # ---------------- Phase A1: q^T -> qT_sb ----------------
                with tc.tile_pool(name="p_a1", bufs=1) as pa1, \
                     tc.tile_pool(name="p_a1x", bufs=2) as pa1x:
                    qw_sb = pa1.tile([P, KT, NH * H], F16, tag="qw")
                    for ch in range(NCH):
                        tsl = slice(ch * CH, (ch + 1) * CH)
                        ps = ps_a.tile([P, NA, CH], F32, tag="qps")
                        for q4 in range(4):
                            if ch == 0:
                                # interleave qw quarters with ch0's x tiles
                                # on the sync FIFO: the first matmuls start
                                # after ~2.3MB instead of ~9MB.
                                nc.sync.dma_start(
                                    qw_sb[:, q4 * XQ:(q4 + 1) * XQ],
                                    qw.ap()[:, q4 * XQ:(q4 + 1) * XQ])
                            xt = pa1x.tile([P, XQ, CH], F16, tag="xt")
                            nc.sync.dma_start(
                                xt[:], xq.ap()[ch, :, q4 * XQ:(q4 + 1) * XQ])
                            for dk in range(XQ):
                                k = q4 * XQ + dk
                                for j in range(NA):
                                    nc.tensor.matmul(
                                        ps[:, j],
                                        qw_sb[:, k, j * P:(j + 1) * P],
                                        xt[:, dk],
                                        start=(k == 0 and j % 2 == 0),
                                        stop=(k == KT - 1),
                                        skip_group_check=True)
                        cs, sn = cos_sb[:, tsl], sin_sb[:, tsl]
                        rope_all(nc.vector, qT_sb[:, :, tsl], ps[:],
                                 cs, sn, ropep, NA // 2)
                        if ch == 0:
                            # gate the k/v-weight prefetch behind ch0: the
                            # copies WRITE into kw_sb/vw_sb, so the DMAs
                            # (WAW) cannot be hoisted ahead of ch0's rope
                            # by the scheduler, keeping the startup loads
                            # uncontended.
                            nc.gpsimd.tensor_copy(kw_sb[0:1, 0, 0:1],
                                                  qT_sb[0:1, 0, 0:1])
                            nc.gpsimd.tensor_copy(vw_sb[0:1, 0, 0:1],
                                                  qT_sb[0:1, 0, 0:1])
                            for q4 in range(4):
                                ksl = slice(q4 * XQ, (q4 + 1) * XQ)
                                nc.gpsimd.dma_start(kw_sb[:, ksl],
                                                    kw.ap()[:, ksl])
                                nc.gpsimd.dma_start(vw_sb[:, ksl],
                                                    vw.ap()[:, ksl])

                # ---------- Phase A2+A3: k^T and v, one xT pass ----------
                with tc.tile_pool(name="p_a2x", bufs=2) as pa2x:
                    for ch in range(NCH):
                        tsl = slice(ch * CH, (ch + 1) * CH)
                        psq = ps_a.tile([P, NA, CH], F32, tag="qps",
                                        name="psq")
                        ps = psq[:, 0:KA]
                        psv = psq[:, KA:].rearrange(
                            "p a c -> p (a c)").rearrange(
                            "p (s h) -> p s h", s=CH // P)
                        for q4 in range(4):
                            xt = pa2x.tile([P, XQ, CH], F16, tag="xt")
                            nc.sync.dma_start(
                                xt[:], xq.ap()[ch, :, q4 * XQ:(q4 + 1) * XQ])
                            for dk in range(XQ):
                                k = q4 * XQ + dk
                                for j in range(KA):
                                    nc.tensor.matmul(
                                        ps[:, j],
                                        kw_sb[:, k, j * P:(j + 1) * P],
                                        xt[:, dk],
                                        start=(k == 0 and j % 2 == 0),
                                        stop=(k == KT - 1),
                                        skip_group_check=True)
                                for st in range(CH // P):
                                    nc.tensor.matmul(
                                        psv[:, st],
                                        xt[:, dk, st * P:(st + 1) * P],
                                        vw_sb[:, k], start=(k == 0),
                                        stop=(k == KT - 1))
                        cs, sn = cos_sb[:, tsl], sin_sb[:, tsl]
                        rope_all(nc.vector, kT_sb[:, :, tsl], ps,
                                 cs, sn, ropep, KA // 2)
                        nc.vector.tensor_copy(
                            v_sb[:, ch * (CH // P):(ch + 1) * (CH // P), :],
                            psv[:])

            # ---------------- Phase B1: attention -> enc_sb ----------
            with tc.tile_pool(name="p_enc", bufs=1) as p_enc:
              enc_sb = p_enc.tile([P, NA, T], BF16)      # 32 KB/part
              # o_w fully resident (56 KB): its loads land in the SBUF the
              # A-phase weight pools just freed, so they stream during B1
              # and B2's stationaries reload only once per (tb, a).
              ow_full = p_enc.tile([P, NA, D], BF16)
              for dch in range(D // 512):
                  nc.scalar.dma_start(
                      ow_full[:, :, dch * 512:(dch + 1) * 512],
                      ow.ap()[:, dch])
              with tc.tile_pool(name="p_b1s", bufs=2) as pb1s, \
                 tc.tile_pool(name="p_b1e", bufs=2) as pb1e, \
                 tc.tile_pool(name="p_sr", bufs=2) as psr, \
                 tc.tile_pool(name="ps_lg", bufs=1, space="PSUM") as ps_lg, \
                 tc.tile_pool(name="ps_sm", bufs=1, space="PSUM") as ps_sm, \
                 tc.tile_pool(name="ps_en", bufs=1, space="PSUM") as ps_en:
                MKJ = {0: 0, 1: 1, 8: 2, 9: 3}

                for kh in range(NKH):
                    for pr in range(T // TW):
                        t0p = pr * TW
                        js = max(0, 8 - 2 * pr)
                        jgroups = []
                        j = js
                        while j < 10:
                            w = min(4, 10 - j)
                            jgroups.append((j, w))
                            j += w
                        exps = [pb1e.tile([P, 10, TW], BF16, tag=f"ex{nl}",
                                           name=f"exps{nl}")
                                for nl in range(2)]
                        smp = [ps_sm.tile([P, TW], F32, tag=f"sm{nl}",
                                         name=f"smp{nl}")
                               for nl in range(2)]
                        encp = [ps_en.tile([P, 2, TW], F32, tag=f"en{nl}",
                                          name=f"encp{nl}")
                               for nl in range(2)]
                        # interleave the two q-heads of this kv head: while
                        # head nl=0 waits on tanh/exp, head nl=1's matmuls
                        # keep the PE busy (and vice versa).
                        for gi, (j0, w) in enumerate(jgroups):
                            for nl in range(2):
                                n = kh * 2 + nl
                                lgT = ps_lg.tile([P, 4, TW], F32,
                                                 tag=f"lg{nl}")
                                for dj in range(w):
                                    j = j0 + dj
                                    s0 = (2 * pr - 8 + j) * P
                                    for hh in range(2):
                                        nc.tensor.matmul(
                                            lgT[:, dj],
                                            kT_sb[:, kh * 2 + hh, s0:s0 + P],
                                            qT_sb[:, 2 * n + hh,
                                                  t0p:t0p + TW],
                                            start=(hh == 0 and dj % 2 == 0),
                                            stop=(hh == 1),
                                            skip_group_check=True)
                                tT = pb1s.tile([P, 4, TW], F32,
                                               tag=f"tT{nl}")
                                # q_w ships unscaled; SCALAR folds in here
                                # (logits enter only via tanh).
                                nc.scalar.activation(
                                    tT[:, :w], lgT[:, :w], Tanh,
                                    scale=SCALAR / SOFT_CAP)
                                for dj in range(w):
                                    j = j0 + dj
                                    if j in MKJ:
                                        nc.vector.tensor_add(
                                            tT[:, dj], tT[:, dj],
                                            mk_sb[:, MKJ[j]])
                                nc.scalar.activation(
                                    exps[nl][:, j0:j0 + w], tT[:, :w],
                                    bias=bias_c[:], func=Exp,
                                    scale=SOFT_CAP)
                                for dj in range(w):
                                    j = j0 + dj
                                    stg = 2 * pr - 8 + j
                                    for hh in range(2):
                                        nc.tensor.matmul(
                                            encp[nl][:, hh],
                                            v_sb[:, stg,
                                                 kh * H + hh * P:
                                                 kh * H + (hh + 1) * P],
                                            exps[nl][:, j],
                                            start=(gi == 0 and dj == 0
                                                   and hh == 0),
                                            stop=(j == 9),
                                            skip_group_check=True)
                        for nl in range(2):
                            # key-sum burst: one 128-col LDW of the all-ones
                            # stationary, then one matmul per key block. The
                            # result rows are identical across partitions —
                            # the broadcast is free.
                            for jj in range(js, 10):
                                nc.tensor.matmul(
                                    smp[nl][:, :], ones_sb[:],
                                    exps[nl][:, jj],
                                    start=(jj == js), stop=(jj == 9),
                                    skip_group_check=True)
                            rbc = psr.tile([P, TW], F32, tag=f"rb{nl}",
                                           name=f"rbc{nl}")
                            nc.vector.reciprocal_approx_fast(
                                rbc[:], smp[nl][:, :])
                            # enc row-tile order: a = kh*4 + nl*2 + hh;
                            # normalization fuses into the PSUM->SBUF copy.
                            for hh in range(2):
                                a = 4 * kh + 2 * nl + hh
                                nc.vector.tensor_mul(
                                    enc_sb[:, a, t0p:t0p + TW],
                                    encp[nl][:, hh], rbc[:])

              # ------------- Phase B2: output projection ---------------
              # enc tiles are the stationaries, loaded once per (tb, a);
              # all 7 d-chunks stream against each (8 LDW per tb instead
              # of 56). po spans 7 banks; the copy is split so tb+1's
              # first accumulation only waits on the first half.
              with tc.tile_pool(name="p_b2o", bufs=2) as pb2o, \
                   tc.tile_pool(name="ps_b2", bufs=2, space="PSUM") as ps_b2:
                    ND = D // 512
                    for tb in range(TB):
                        t0 = tb * P
                        stage = pb2o.tile([P, ND, 512], F16, tag="ob")
                        po = ps_b2.tile([P, 4, 512], F32, tag="po")
                        for a in range(NA):
                            for dch in range(4):
                                nc.tensor.matmul(
                                    po[:, dch], enc_sb[:, a, t0:t0 + P],
                                    ow_full[:, a,
                                            dch * 512:(dch + 1) * 512],
                                    start=(a == 0), stop=(a == NA - 1),
                                    skip_group_check=True)
                        nc.scalar.copy(stage[:, 0:4], po[:])
                        po2 = ps_b2.tile([P, 4, 512], F32, tag="po")
                        for a in range(NA):
                            for dch in range(4, ND):
                                nc.tensor.matmul(
                                    po2[:, dch - 4],
                                    enc_sb[:, a, t0:t0 + P],
                                    ow_full[:, a,
                                            dch * 512:(dch + 1) * 512],
                                    start=(a == 0), stop=(a == NA - 1),
                                    skip_group_check=True)
                        nc.scalar.copy(stage[:, 4:], po2[:, 0:3])
                        nc.sync.dma_start(out_p.ap()[tb], stage[:])

    nc.compile()
    return nc


def _get_program():
    if "nc" not in _PROG_CACHE:
        _PROG_CACHE["nc"] = _build_program()
    return _PROG_CACHE["nc"]


def _host_inputs(x, segment_pos, q_w, kv_w, o_w):
    """Build the 8 per-core input dicts. All large operands ship as fp16."""
    BF = np.float16
    # x: [T, D] -> [ch, p, ktile, c] so each partition's tile-slice is one
    # contiguous DRAM run (see kernel layout note).
    xqs = [np.ascontiguousarray(
        x[b].astype(BF).reshape(NCH_H, CH_H, KT, P).transpose(0, 3, 2, 1))
        for b in range(B)]
    tabs = []
    for b in range(B):
        pos = segment_pos[b].astype(np.float64)
        inv_ts = BASE ** (-2.0 * np.arange(H // 2, dtype=np.float64) / H)
        ang = inv_ts[:, None] * pos[None, :]          # [128, T]
        tabs.append((np.cos(ang).astype(BF), np.sin(ang).astype(BF)))

    i = np.arange(P)[:, None]
    c = np.arange(TW)[None, :]
    tiles = []
    for j in (0, 1, 8, 9):
        valid = (c >= P * j + i - WINDOW) & (c <= P * j + i - 1)
        tiles.append(np.where(valid, np.float32(0.0), np.float32(NEG)))
    maskt = np.concatenate(tiles, axis=0).astype(BF)

    in_maps = []
    for core in range(NCORES):
        b, hh = divmod(core, 4)
        # q-heads [4*hh, +4), kv-heads [2*hh, +2)
        qws = np.ascontiguousarray(
            q_w[4 * hh:4 * hh + 4].transpose(1, 0, 2).reshape(
                KT, P, NH * H).transpose(1, 0, 2)).astype(BF)
        kws = np.ascontiguousarray(
            kv_w[0, 2 * hh:2 * hh + 2].transpose(1, 0, 2).reshape(
                KT, P, NKH * H).transpose(1, 0, 2)).astype(BF)
        vws = np.ascontiguousarray(
            kv_w[1, 2 * hh:2 * hh + 2].transpose(1, 0, 2).reshape(
                KT, P, NKH * H).transpose(1, 0, 2)).astype(BF)
        # row-tile order a = kh*4 + nl*2 + hh2 matching B1 writes
        ow_tiles = []
        for a in range(NA):
            kh, r = divmod(a, 4)
            nl, hh2 = divmod(r, 2)
            ow_tiles.append(
                o_w[4 * hh + 2 * kh + nl, hh2 * P:(hh2 + 1) * P, :])
        import ml_dtypes
        ows = np.concatenate(ow_tiles, axis=0)          # [NA*P, D]
        ows = np.ascontiguousarray(
            ows.reshape(NA, P, D // 512, 512).transpose(1, 2, 0, 3)).astype(
            ml_dtypes.bfloat16)
        in_maps.append({
            "xq": xqs[b], "qw": qws, "kw": kws, "vw": vws, "ow": ows,
            "cosT": tabs[b][0], "sinT": tabs[b][1], "maskt": maskt,
        })
    return in_maps


def kernel(x, segment_pos, attn_mask, q_w, kv_w, o_w):
    from concourse import bass_utils

    x = np.asarray(x, dtype=np.float32)
    q_w = np.asarray(q_w, dtype=np.float32)
    kv_w = np.asarray(kv_w, dtype=np.float32)
    o_w = np.asarray(o_w, dtype=np.float32)
    segment_pos = np.asarray(segment_pos)

    nc = _get_program()
    in_maps = _host_inputs(x, segment_pos, q_w, kv_w, o_w)
    res = bass_utils.run_bass_kernel_spmd(nc, in_maps,
                                          core_ids=list(range(NCORES)))
    out = np.zeros((B, T, D), dtype=np.float32)
    for core in range(NCORES):
        part = res.results[core]["out_p"].astype(np.float32)
        out[core // 4] += part.reshape(T, D)
    return out


# revision 28
# speedup vs baseline: 1.0155x; 1.0155x over previous
"""Sliding-window GQA attention (RoPE + tanh soft-cap) on 8 Trainium2 cores.

Sharding: core c = 4*b + hh with b = batch, hh = head-quarter. Each core
handles batch b, q-heads [4*hh, 4*hh+4), kv-heads [2*hh, 2*hh+2) — one
head-group of (4 q-heads, 2 kv-heads); the host sums the 4 partials per batch.

Phases per core:
  A1: q^T = (q_w^T @ x^T), RoPE              -> SBUF qT_sb [128, 8, 2048] f16
  A2: k^T = (k_w^T @ x^T), RoPE              -> SBUF [512, 2048] f16
  A3: v   = (x @ v_w)                        -> SBUF [2048, 512] bf16
  B1: transposed-band attention              -> enc_sb rows [0, 8)
  B2: out = enc^T.T @ o_w over 8 row-tiles   -> DRAM [2048, 3584] partial.

Ring assignment: sync = qw + x stream + output stage; scalar = tables,
sum-row spill, ow loads; gpsimd = kw/vw prefetch (overlaps A1) + sum-row
broadcast. The per-head normalization runs on GpSimd so the slow
DRAM-broadcast round trip never blocks the Vector FIFO. B1 interleaves the
two q-heads of each kv head so one head's softcap/exp latency hides under
the other head's matmuls.
"""

import numpy as np

B, T, D, N, KH, H = 2, 2048, 3584, 16, 8, 256
WINDOW = 1024
SOFT_CAP = 50.0
SCALAR = 0.0625
BASE = 10000.0
NEG = -30000.0  # tanh-domain mask (fits fp16; exp(50*NEG) underflows to 0)

P = 128
NH = 4         # q heads per core
NKH = 2        # kv heads per core
KT = D // P    # 28 contraction tiles
NA = NH * (H // P)   # 8 q^T row-tiles per core
KA = NKH * (H // P)  # 4 k^T row-tiles per core
TB = T // P    # 16 query blocks
TW = 256       # query-pair width for the transposed-attention phase
NCORES = 8
NCH_H = 8      # projection t-chunks (host mirror of NCH)
CH_H = 256

_PROG_CACHE = {}


def _build_program():
    import concourse.bacc as bacc
    import concourse.tile as tile
    import concourse.mybir as mybir

    F32 = mybir.dt.float32
    F16 = mybir.dt.float16
    BF16 = mybir.dt.bfloat16
    Tanh = mybir.ActivationFunctionType.Tanh
    Exp = mybir.ActivationFunctionType.Exp

    nc = bacc.Bacc("TRN2", target_bir_lowering=False, debug=False,
                   num_devices=NCORES)

    CH = 256             # t-chunk for projections
    NCH = T // CH        # 8

    # All DRAM operands are laid out so each SBUF partition's slice is one
    # contiguous DRAM run (HWDGE descriptor generation is the DMA
    # bottleneck otherwise: a [D,T]-layout x gives 512B runs, ~900
    # descriptors and 8-15us per tile load).
    xq = nc.dram_tensor("xq", [NCH, P, KT, CH], F16, kind="ExternalInput")
    qw = nc.dram_tensor("qw", [P, KT, NH * H], F16, kind="ExternalInput")
    kw = nc.dram_tensor("kw", [P, KT, NKH * H], F16, kind="ExternalInput")
    vw = nc.dram_tensor("vw", [P, KT, NKH * H], F16, kind="ExternalInput")
    ow = nc.dram_tensor("ow", [P, D // 512, NA, 512], BF16,
                        kind="ExternalInput")
    cosT = nc.dram_tensor("cosT", [P, T], F16, kind="ExternalInput")
    sinT = nc.dram_tensor("sinT", [P, T], F16, kind="ExternalInput")
    maskt = nc.dram_tensor("maskt", [4 * P, TW], F16, kind="ExternalInput")
    out_p = nc.dram_tensor("out_p", [TB, P, D], F16,
                           kind="ExternalOutput")

    def rope_all(vec, dst, src, cs, sn, tmp_pool, npair):
        # all pairs at once via strided even/odd views:
        # even' = even*cos - odd*sin ; odd' = odd*cos + even*sin
        sv = src.rearrange("p (a x) c -> p x a c", x=2)
        dv = dst.rearrange("p (a x) c -> p x a c", x=2)
        se, so = sv[:, 0], sv[:, 1]
        de, do = dv[:, 0], dv[:, 1]
        csb = cs.unsqueeze(1).broadcast_to([P, npair, CH])
        snb = sn.unsqueeze(1).broadcast_to([P, npair, CH])
        t1 = tmp_pool.tile([P, NA // 2, CH], F16, tag="rt1",
                           name="t1")[:, :npair]
        t2 = tmp_pool.tile([P, NA // 2, CH], F16, tag="rt2",
                           name="t2")[:, :npair]
        vec.tensor_mul(t1, se, csb)
        vec.tensor_mul(t2, so, snb)
        vec.tensor_sub(de, t1, t2)
        # in-order DVE execution makes the tmp reuse safe
        vec.tensor_mul(t1, so, csb)
        vec.tensor_mul(t2, se, snb)
        vec.tensor_add(do, t1, t2)

    import concourse.bass as bass_mod

    with tile.TileContext(nc) as tc:
        with tc.tile_pool(name="p_tab", bufs=1) as p_tab, \
             tc.tile_pool(name="p_perm", bufs=1) as p_perm:
            # tables ride the scalar HWDGE ring: they must not queue ahead
            # of the weight/x loads on the sync ring.
            cos_sb = p_tab.tile([P, T], F16)
            sin_sb = p_tab.tile([P, T], F16)
            nc.scalar.dma_start(cos_sb[:], cosT.ap())
            nc.scalar.dma_start(sin_sb[:], sinT.ap())
            mk_sb = p_tab.tile([P, 4, TW], F16)
            nc.scalar.dma_start(mk_sb[:], maskt.ap().rearrange(
                "(m p) c -> p m c", p=P))
            bias_c = p_tab.tile([P, 1], F32)
            nc.vector.memset(bias_c[:], -10.0)
            # all-ones [128,128] stationary: the key-sum matmul then lands
            # the per-query sums replicated across all 128 partitions, so
            # normalization needs no DRAM broadcast round trip.
            ones_sb = p_tab.tile([P, P], BF16)
            nc.vector.memset(ones_sb[:], 1.0)

            qT_sb = p_perm.tile([P, NA, T], F16)        # 32 KB/part
            kT_sb = p_perm.tile([P, KA, T], F16)        # 16 KB/part
            v_sb = p_perm.tile([P, TB, NKH * H], BF16)  # 16 KB/part

            with tc.tile_pool(name="p_wkv", bufs=1) as p_wkv, \
                 tc.tile_pool(name="ps_a", bufs=2, space="PSUM") as ps_a, \
                 tc.tile_pool(name="p_rope", bufs=2) as ropep:
                # k/v weights prefetch during A1 on the scalar ring (queued
                # behind the small tables), into SBUF disjoint from qw so
                # there is no WAR delay at the A1->A2 boundary and no SDMA
                # contention with the sync ring's critical qw/x loads.
                kw_sb = p_wkv.tile([P, KT, NKH * H], F16, tag="kw")
                vw_sb = p_wkv.tile([P, KT, NKH * H], F16, tag="vw")
                XQ = KT // 4

                # ---------------- Phase A1: q^T -> qT_sb ----------------
                with tc.tile_pool(name="p_a1", bufs=1) as pa1, \
                     tc.tile_pool(name="p_a1x", bufs=2) as pa1x:
                    qw_sb = pa1.tile([P, KT, NH * H], F16, tag="qw")
                    for ch in range(NCH):
                        tsl = slice(ch * CH, (ch + 1) * CH)
                        ps = ps_a.tile([P, NA, CH], F32, tag="qps")
                        for q4 in range(4):
                            if ch == 0:
                                # split qw across the sync and gpsimd rings
                                # and interleave with ch0's x tiles: doubles
                                # effective startup load bandwidth.
                                eng = nc.sync if q4 % 2 == 0 else nc.gpsimd
                                eng.dma_start(
                                    qw_sb[:, q4 * XQ:(q4 + 1) * XQ],
                                    qw.ap()[:, q4 * XQ:(q4 + 1) * XQ])
                            xt = pa1x.tile([P, XQ, CH], F16, tag="xt")
                            nc.sync.dma_start(
                                xt[:], xq.ap()[ch, :, q4 * XQ:(q4 + 1) * XQ])
                            for dk in range(XQ):
                                k = q4 * XQ + dk
                                for j in range(NA):
                                    nc.tensor.matmul(
                                        ps[:, j],
                                        qw_sb[:, k, j * P:(j + 1) * P],
                                        xt[:, dk],
                                        start=(k == 0 and j % 2 == 0),
                                        stop=(k == KT - 1),
                                        skip_group_check=True)
                        cs, sn = cos_sb[:, tsl], sin_sb[:, tsl]
                        rope_all(nc.vector, qT_sb[:, :, tsl], ps[:],
                                 cs, sn, ropep, NA // 2)
                        if ch == 0:
                            # gate the k/v-weight prefetch behind ch0: the
                            # copies WRITE into kw_sb/vw_sb, so the DMAs
                            # (WAW) cannot be hoisted ahead of ch0's rope
                            # by the scheduler, keeping the startup loads
                            # uncontended.
                            nc.gpsimd.tensor_copy(kw_sb[0:1, 0, 0:1],
                                                  qT_sb[0:1, 0, 0:1])
                            nc.gpsimd.tensor_copy(vw_sb[0:1, 0, 0:1],
                                                  qT_sb[0:1, 0, 0:1])
                            for q4 in range(4):
                                ksl = slice(q4 * XQ, (q4 + 1) * XQ)
                                nc.gpsimd.dma_start(kw_sb[:, ksl],
                                                    kw.ap()[:, ksl])
                                nc.gpsimd.dma_start(vw_sb[:, ksl],
                                                    vw.ap()[:, ksl])

                # ---------- Phase A2+A3: k^T and v, one xT pass ----------
                with tc.tile_pool(name="p_a2x", bufs=2) as pa2x:
                    for ch in range(NCH):
                        tsl = slice(ch * CH, (ch + 1) * CH)
                        psq = ps_a.tile([P, NA, CH], F32, tag="qps",
                                        name="psq")
                        ps = psq[:, 0:KA]
                        psv = psq[:, KA:].rearrange(
                            "p a c -> p (a c)").rearrange(
                            "p (s h) -> p s h", s=CH // P)
                        for q4 in range(4):
                            xt = pa2x.tile([P, XQ, CH], F16, tag="xt")
                            nc.sync.dma_start(
                                xt[:], xq.ap()[ch, :, q4 * XQ:(q4 + 1) * XQ])
                            for dk in range(XQ):
                                k = q4 * XQ + dk
                                for j in range(KA):
                                    nc.tensor.matmul(
                                        ps[:, j],
                                        kw_sb[:, k, j * P:(j + 1) * P],
                                        xt[:, dk],
                                        start=(k == 0 and j % 2 == 0),
                                        stop=(k == KT - 1),
                                        skip_group_check=True)
                                for st in range(CH // P):
                                    nc.tensor.matmul(
                                        psv[:, st],
                                        xt[:, dk, st * P:(st + 1) * P],
                                        vw_sb[:, k], start=(k == 0),
                                        stop=(k == KT - 1))
                        cs, sn = cos_sb[:, tsl], sin_sb[:, tsl]
                        rope_all(nc.vector, kT_sb[:, :, tsl], ps,
                                 cs, sn, ropep, KA // 2)
                        nc.vector.tensor_copy(
                            v_sb[:, ch * (CH // P):(ch + 1) * (CH // P), :],
                            psv[:])

            # ---------------- Phase B1: attention -> enc_sb ----------
            with tc.tile_pool(name="p_enc", bufs=1) as p_enc:
              enc_sb = p_enc.tile([P, NA, T], BF16)      # 32 KB/part
              # o_w fully resident (56 KB): its loads land in the SBUF the
              # A-phase weight pools just freed, so they stream during B1
              # and B2's stationaries reload only once per (tb, a).
              ow_full = p_enc.tile([P, NA, D], BF16)
              for dch in range(D // 512):
                  # gpsimd (SWDGE) ring: idle during B1, and its triggers
                  # don't block — scalar-ring triggers would stall B1's
                  # first tanh behind 7MB of weight traffic.
                  nc.gpsimd.dma_start(
                      ow_full[:, :, dch * 512:(dch + 1) * 512],
                      ow.ap()[:, dch])
              with tc.tile_pool(name="p_b1s", bufs=2) as pb1s, \
                 tc.tile_pool(name="p_b1e", bufs=2) as pb1e, \
                 tc.tile_pool(name="p_sr", bufs=2) as psr, \
                 tc.tile_pool(name="ps_lg", bufs=1, space="PSUM") as ps_lg, \
                 tc.tile_pool(name="ps_sm", bufs=1, space="PSUM") as ps_sm, \
                 tc.tile_pool(name="ps_en", bufs=1, space="PSUM") as ps_en:
                MKJ = {0: 0, 1: 1, 8: 2, 9: 3}

                for kh in range(NKH):
                    for pr in range(T // TW):
                        t0p = pr * TW
                        js = max(0, 8 - 2 * pr)
                        jgroups = []
                        j = js
                        while j < 10:
                            w = min(4, 10 - j)
                            jgroups.append((j, w))
                            j += w
                        exps = [pb1e.tile([P, 10, TW], BF16, tag=f"ex{nl}",
                                           name=f"exps{nl}")
                                for nl in range(2)]
                        smp = [ps_sm.tile([P, TW], F32, tag=f"sm{nl}",
                                         name=f"smp{nl}")
                               for nl in range(2)]
                        encp = [ps_en.tile([P, 2, TW], F32, tag=f"en{nl}",
                                          name=f"encp{nl}")
                               for nl in range(2)]
                        # interleave the two q-heads of this kv head: while
                        # head nl=0 waits on tanh/exp, head nl=1's matmuls
                        # keep the PE busy (and vice versa).
                        for gi, (j0, w) in enumerate(jgroups):
                            for nl in range(2):
                                n = kh * 2 + nl
                                lgT = ps_lg.tile([P, 4, TW], F32,
                                                 tag=f"lg{nl}")
                                for dj in range(w):
                                    j = j0 + dj
                                    s0 = (2 * pr - 8 + j) * P
                                    for hh in range(2):
                                        nc.tensor.matmul(
                                            lgT[:, dj],
                                            kT_sb[:, kh * 2 + hh, s0:s0 + P],
                                            qT_sb[:, 2 * n + hh,
                                                  t0p:t0p + TW],
                                            start=(hh == 0 and dj % 2 == 0),
                                            stop=(hh == 1),
                                            skip_group_check=True)
                                tT = pb1s.tile([P, 4, TW], F32,
                                               tag=f"tT{nl}")
                                # q_w ships unscaled; SCALAR folds in here
                                # (logits enter only via tanh).
                                nc.scalar.activation(
                                    tT[:, :w], lgT[:, :w], Tanh,
                                    scale=SCALAR / SOFT_CAP)
                                for dj in range(w):
                                    j = j0 + dj
                                    if j in MKJ:
                                        nc.vector.tensor_add(
                                            tT[:, dj], tT[:, dj],
                                            mk_sb[:, MKJ[j]])
                                nc.scalar.activation(
                                    exps[nl][:, j0:j0 + w], tT[:, :w],
                                    bias=bias_c[:], func=Exp,
                                    scale=SOFT_CAP)
                                for dj in range(w):
                                    j = j0 + dj
                                    stg = 2 * pr - 8 + j
                                    for hh in range(2):
                                        nc.tensor.matmul(
                                            encp[nl][:, hh],
                                            v_sb[:, stg,
                                                 kh * H + hh * P:
                                                 kh * H + (hh + 1) * P],
                                            exps[nl][:, j],
                                            start=(gi == 0 and dj == 0
                                                   and hh == 0),
                                            stop=(j == 9),
                                            skip_group_check=True)
                        for nl in range(2):
                            # key-sum burst: one 128-col LDW of the all-ones
                            # stationary, then one matmul per key block. The
                            # result rows are identical across partitions —
                            # the broadcast is free.
                            for jj in range(js, 10):
                                nc.tensor.matmul(
                                    smp[nl][:, :], ones_sb[:],
                                    exps[nl][:, jj],
                                    start=(jj == js), stop=(jj == 9),
                                    skip_group_check=True)
                            rbc = psr.tile([P, TW], F32, tag=f"rb{nl}",
                                           name=f"rbc{nl}")
                            nc.vector.reciprocal_approx_fast(
                                rbc[:], smp[nl][:, :])
                            # enc row-tile order: a = kh*4 + nl*2 + hh;
                            # normalization fuses into the PSUM->SBUF copy.
                            for hh in range(2):
                                a = 4 * kh + 2 * nl + hh
                                nc.vector.tensor_mul(
                                    enc_sb[:, a, t0p:t0p + TW],
                                    encp[nl][:, hh], rbc[:])

              # ------------- Phase B2: output projection ---------------
              # enc tiles are the stationaries, loaded once per (tb, a);
              # all 7 d-chunks stream against each (8 LDW per tb instead
              # of 56). po spans 7 banks; the copy is split so tb+1's
              # first accumulation only waits on the first half.
              with tc.tile_pool(name="p_b2o", bufs=2) as pb2o, \
                   tc.tile_pool(name="ps_b2", bufs=2, space="PSUM") as ps_b2:
                    ND = D // 512
                    for tb in range(TB):
                        t0 = tb * P
                        stage = pb2o.tile([P, ND, 512], F16, tag="ob")
                        po = ps_b2.tile([P, 4, 512], F32, tag="po")
                        for a in range(NA):
                            for dch in range(4):
                                nc.tensor.matmul(
                                    po[:, dch], enc_sb[:, a, t0:t0 + P],
                                    ow_full[:, a,
                                            dch * 512:(dch + 1) * 512],
                                    start=(a == 0), stop=(a == NA - 1),
                                    skip_group_check=True)
                        nc.scalar.copy(stage[:, 0:4], po[:])
                        po2 = ps_b2.tile([P, 4, 512], F32, tag="po")
                        for a in range(NA):
                            for dch in range(4, ND):
                                nc.tensor.matmul(
                                    po2[:, dch - 4],
                                    enc_sb[:, a, t0:t0 + P],
                                    ow_full[:, a,
                                            dch * 512:(dch + 1) * 512],
                                    start=(a == 0), stop=(a == NA - 1),
                                    skip_group_check=True)
                        nc.scalar.copy(stage[:, 4:], po2[:, 0:3])
                        nc.sync.dma_start(out_p.ap()[tb], stage[:])

    nc.compile()
    return nc


def _get_program():
    if "nc" not in _PROG_CACHE:
        _PROG_CACHE["nc"] = _build_program()
    return _PROG_CACHE["nc"]


def _host_inputs(x, segment_pos, q_w, kv_w, o_w):
    """Build the 8 per-core input dicts. All large operands ship as fp16."""
    BF = np.float16
    # x: [T, D] -> [ch, p, ktile, c] so each partition's tile-slice is one
    # contiguous DRAM run (see kernel layout note).
    xqs = [np.ascontiguousarray(
        x[b].astype(BF).reshape(NCH_H, CH_H, KT, P).transpose(0, 3, 2, 1))
        for b in range(B)]
    tabs = []
    for b in range(B):
        pos = segment_pos[b].astype(np.float64)
        inv_ts = BASE ** (-2.0 * np.arange(H // 2, dtype=np.float64) / H)
        ang = inv_ts[:, None] * pos[None, :]          # [128, T]
        tabs.append((np.cos(ang).astype(BF), np.sin(ang).astype(BF)))

    i = np.arange(P)[:, None]
    c = np.arange(TW)[None, :]
    tiles = []
    for j in (0, 1, 8, 9):
        valid = (c >= P * j + i - WINDOW) & (c <= P * j + i - 1)
        tiles.append(np.where(valid, np.float32(0.0), np.float32(NEG)))
    maskt = np.concatenate(tiles, axis=0).astype(BF)

    in_maps = []
    for core in range(NCORES):
        b, hh = divmod(core, 4)
        # q-heads [4*hh, +4), kv-heads [2*hh, +2)
        qws = np.ascontiguousarray(
            q_w[4 * hh:4 * hh + 4].transpose(1, 0, 2).reshape(
                KT, P, NH * H).transpose(1, 0, 2)).astype(BF)
        kws = np.ascontiguousarray(
            kv_w[0, 2 * hh:2 * hh + 2].transpose(1, 0, 2).reshape(
                KT, P, NKH * H).transpose(1, 0, 2)).astype(BF)
        vws = np.ascontiguousarray(
            kv_w[1, 2 * hh:2 * hh + 2].transpose(1, 0, 2).reshape(
                KT, P, NKH * H).transpose(1, 0, 2)).astype(BF)
        # row-tile order a = kh*4 + nl*2 + hh2 matching B1 writes
        ow_tiles = []
        for a in range(NA):
            kh, r = divmod(a, 4)
            nl, hh2 = divmod(r, 2)
            ow_tiles.append(
                o_w[4 * hh + 2 * kh + nl, hh2 * P:(hh2 + 1) * P, :])
        import ml_dtypes
        ows = np.concatenate(ow_tiles, axis=0)          # [NA*P, D]
        ows = np.ascontiguousarray(
            ows.reshape(NA, P, D // 512, 512).transpose(1, 2, 0, 3)).astype(
            ml_dtypes.bfloat16)
        in_maps.append({
            "xq": xqs[b], "qw": qws, "kw": kws, "vw": vws, "ow": ows,
            "cosT": tabs[b][0], "sinT": tabs[b][1], "maskt": maskt,
        })
    return in_maps


def kernel(x, segment_pos, attn_mask, q_w, kv_w, o_w):
    from concourse import bass_utils

    x = np.asarray(x, dtype=np.float32)
    q_w = np.asarray(q_w, dtype=np.float32)
    kv_w = np.asarray(kv_w, dtype=np.float32)
    o_w = np.asarray(o_w, dtype=np.float32)
    segment_pos = np.asarray(segment_pos)

    nc = _get_program()
    in_maps = _host_inputs(x, segment_pos, q_w, kv_w, o_w)
    res = bass_utils.run_bass_kernel_spmd(nc, in_maps,
                                          core_ids=list(range(NCORES)))
    out = np.zeros((B, T, D), dtype=np.float32)
    for core in range(NCORES):
        part = res.results[core]["out_p"].astype(np.float32)
        out[core // 4] += part.reshape(T, D)
    return out
